# revision 7
# baseline (speedup 1.0000x reference)
"""Trainium2 Bass kernel for multi-head attention + residual + LayerNorm.

Problem: B=4, L=1024, D=1024, H=16, DK=DV=64, causal mask, returns
(out, attn) like the reference:
    qh = q @ Wq.T (split heads), kh = k @ Wk.T, vh = v @ Wv.T
    attn = softmax(mask(qh kh^T / sqrt(DK)))
    o = attn @ vh  (merge heads) + q ; out = LayerNorm(o) * gamma + beta

Sharding (8 cores): kernel 1 is head-parallel — core c handles batch c//2
and heads (c%2)*8..(c%2)*8+8, full query range. Every core runs an
identical program (causal structure is the same on all cores). Kernel 2
does residual+LayerNorm, row-parallel (512 rows per core).

Layout strategy (per core, kernel 1):
  - host passes transposed qT/kT/vT [D, L] and wqT/wkT/wvT [D, 512] so all
    projection matmuls contract d on partitions.
  - projections produce qhT/khT [e, lq] (e on partitions) and vh [lk, e].
  - natural scores [lq, lk] per (head, lq-tile): K=64 matmul from qhT/khT
    slices; causal -1e9 bias added on the diagonal 128-block; exp on
    ScalarE with fused row-sum (accum_out); normalize with per-partition
    reciprocal via tensor_scalar; DMA straight to the attn output.
  - transposed scores [lk, lq] are recomputed (cheaper than transposing
    attn through PSUM) for the attn^T @ v contraction; o comes out as
    oT [64, lq] accumulated over lk tiles, PE-transposed back and scaled
    by the same reciprocals.
Causality is exploited everywhere: upper-triangle blocks are never
computed; the attn output buffer is pre-zeroed by the runtime.
"""

import math
from contextlib import ExitStack

import numpy as np

import concourse.bass as bass
import concourse.tile as tile
from concourse import mybir
import concourse.bass_utils as bass_utils

F32 = mybir.dt.float32
AF = mybir.ActivationFunctionType

B = 4
L = 1024
D = 1024
H = 16
DK = 64
HH = H // 2          # heads per core (head-half)
EH = HH * DK         # 512 output dims per core
NT = L // 128        # 8 row tiles
ND = D // 128        # 8 contraction tiles
NE = EH // 128       # 4 e-tiles per core
N_CORES = 8
NEG = -1.0e9
LN_EPS = 1e-5
SCALE = 1.0 / math.sqrt(DK)


def _split_multiwaits(nc, max_waits=1):
    """walrus in this toolchain rejects >1 sync-wait per instruction; move
    extra waits onto NoOps inserted right before (same engine, same order —
    semantically identical)."""
    uid = [0]
    for func in nc.m.functions:
        for block in func.blocks:
            new_insts = []
            for inst in block.instructions:
                si = inst.sync_info
                if si is not None and si.on_wait and len(si.on_wait) > max_waits:
                    waits = list(si.on_wait)
                    for w in waits[:-max_waits]:
                        uid[0] += 1
                        nop = mybir.InstNoOp(
                            name=f"I-waitsplit-{uid[0]}",
                            engine=inst.engine,
                            sync_info=mybir.SyncInfo(on_wait=[w], on_update=[]),
                            ins=[],
                            outs=[],
                        )
                        new_insts.append(nop)
                        nc.inst_map[nop.name] = nop
                    inst.sync_info = mybir.SyncInfo(
                        on_wait=waits[-max_waits:], on_update=list(si.on_update or [])
                    )
                new_insts.append(inst)
            block.instructions[:] = new_insts


def _build_attn_program():
    nc = bass.Bass()
    qT = nc.dram_tensor("qT", [D, L], F32, kind="ExternalInput")
    kT = nc.dram_tensor("kT", [D, L], F32, kind="ExternalInput")
    vT = nc.dram_tensor("vT", [D, L], F32, kind="ExternalInput")
    wqT = nc.dram_tensor("wqT", [D, EH], F32, kind="ExternalInput")
    wkT = nc.dram_tensor("wkT", [D, EH], F32, kind="ExternalInput")
    wvT = nc.dram_tensor("wvT", [D, EH], F32, kind="ExternalInput")
    triu = nc.dram_tensor("triu", [128, 128], F32, kind="ExternalInput")
    trilT = nc.dram_tensor("trilT", [128, 128], F32, kind="ExternalInput")
    ident = nc.dram_tensor("ident", [128, 128], F32, kind="ExternalInput")
    attn_p = nc.dram_tensor("attn_p", [HH, L, L], F32, kind="ExternalOutput")
    o_p = nc.dram_tensor("o_p", [L, EH], F32, kind="ExternalOutput")

    with ExitStack() as ctx:
        tc = ctx.enter_context(tile.TileContext(nc))
        const = ctx.enter_context(tc.tile_pool(name="const", bufs=1))
        w_pool = ctx.enter_context(tc.tile_pool(name="w", bufs=9))
        x_pool = ctx.enter_context(tc.tile_pool(name="x", bufs=9))
        qhT_pool = ctx.enter_context(tc.tile_pool(name="qhT", bufs=NE))
        khT_pool = ctx.enter_context(tc.tile_pool(name="khT", bufs=NE))
        vh_pool = ctx.enter_context(tc.tile_pool(name="vh", bufs=NT))
        expn_pool = ctx.enter_context(tc.tile_pool(name="expn", bufs=HH + 1))
        expt_pool = ctx.enter_context(tc.tile_pool(name="expt", bufs=3))
        osb_pool = ctx.enter_context(tc.tile_pool(name="osb", bufs=NT))
        ot_pool = ctx.enter_context(tc.tile_pool(name="ot", bufs=2))
        stat_pool = ctx.enter_context(tc.tile_pool(name="stat", bufs=2))
        rec_pool = ctx.enter_context(tc.tile_pool(name="rec", bufs=NT))
        ps_s = ctx.enter_context(tc.tile_pool(name="ps_s", bufs=2, space="PSUM"))
        ps_o = ctx.enter_context(tc.tile_pool(name="ps_o", bufs=1, space="PSUM"))
        ps_t = ctx.enter_context(tc.tile_pool(name="ps_t", bufs=2, space="PSUM"))

        triu_t = const.tile([128, 128], F32)
        nc.sync.dma_start(out=triu_t, in_=triu[:, :])
        trilT_t = const.tile([128, 128], F32)
        nc.sync.dma_start(out=trilT_t, in_=trilT[:, :])
        ident_t = const.tile([128, 128], F32)
        nc.sync.dma_start(out=ident_t, in_=ident[:, :])

        # ---- projections -------------------------------------------------
        def project(w_dram, x_dram, n_out_tiles, transposed_out):
            """transposed_out=True: produce [e, l] tiles (accumulate lhsT=w
            slices); False: produce [l_tile, e] tiles (lhsT=x slices)."""
            wt = []
            xt = []
            for dt in range(ND):
                w_t = w_pool.tile([128, EH], F32, tag="w")
                nc.sync.dma_start(out=w_t, in_=w_dram[dt * 128:(dt + 1) * 128, :])
                wt.append(w_t)
                x_t = x_pool.tile([128, L], F32, tag="x")
                nc.sync.dma_start(out=x_t, in_=x_dram[dt * 128:(dt + 1) * 128, :])
                xt.append(x_t)
            out_tiles = []
            if transposed_out:
                # out[e_tile][128, L]
                for et in range(NE):
                    ot = None
                    for lqc in range(L // 512):
                        ps = ps_s.tile([128, L], F32, tag="sc")
                        for dt in range(ND):
                            nc.tensor.matmul(
                                ps[:, 0:512],
                                lhsT=wt[dt][:, et * 128:(et + 1) * 128],
                                rhs=xt[dt][:, lqc * 512:(lqc + 1) * 512],
                                start=(dt == 0),
                                stop=(dt == ND - 1),
                            )
                        if ot is None:
                            ot = qhT_pool.tile([128, L], F32, tag=transposed_out)
                        nc.scalar.copy(
                            out=ot[:, lqc * 512:(lqc + 1) * 512], in_=ps[:, 0:512]
                        )
                    out_tiles.append(ot)
            else:
                # out[lk_tile][128, EH]
                for lkt in range(NT):
                    ps = ps_s.tile([128, L], F32, tag="sc")
                    for dt in range(ND):
                        nc.tensor.matmul(
                            ps[:, 0:EH],
                            lhsT=xt[dt][:, lkt * 128:(lkt + 1) * 128],
                            rhs=wt[dt][:, 0:EH],
                            start=(dt == 0),
                            stop=(dt == ND - 1),
                        )
                    vt = vh_pool.tile([128, EH], F32, tag="vh")
                    nc.scalar.copy(out=vt, in_=ps[:, 0:EH])
                    out_tiles.append(vt)
            return out_tiles

        qhT_t = project(wqT, qT, NE, "qhT")
        khT_t = project(wkT, kT, NE, "khT")
        vh_t = project(wvT, vT, NT, False)

        # ---- natural-layout pass: attn output + row sums ----------------
        recs = []
        exp_nat = {}
        for t in range(NT):
            W = 128 * (t + 1)
            sum_t = stat_pool.tile([128, HH], F32, tag="sums")
            exp_tiles = []
            for h in range(HH):
                et, hr = h // 2, (h % 2) * 64
                ps = ps_s.tile([128, L], F32, tag="sc")
                for c0 in range(0, W, 512):
                    N = min(512, W - c0)
                    nc.tensor.matmul(
                        ps[:, c0:c0 + N],
                        lhsT=qhT_t[et][hr:hr + 64, t * 128:(t + 1) * 128],
                        rhs=khT_t[et][hr:hr + 64, c0:c0 + N],
                        start=True,
                        stop=True,
                    )
                nc.vector.tensor_add(
                    ps[:, t * 128:(t + 1) * 128],
                    ps[:, t * 128:(t + 1) * 128],
                    triu_t,
                )
                ex = expn_pool.tile([128, L], F32, tag="expn")
                nc.scalar.activation(
                    out=ex[:, 0:W],
                    in_=ps[:, 0:W],
                    func=AF.Exp,
                    scale=SCALE,
                    accum_out=sum_t[:, h:h + 1],
                )
                exp_tiles.append(ex)
            rec_t = rec_pool.tile([128, HH], F32, tag="rec")
            nc.vector.reciprocal(rec_t, sum_t)
            recs.append(rec_t)
            for h in range(HH):
                ex = exp_tiles[h]
                nc.vector.tensor_scalar_mul(ex[:, 0:W], ex[:, 0:W], rec_t[:, h:h + 1])
                nc.sync.dma_start(
                    out=attn_p[h, t * 128:(t + 1) * 128, 0:W], in_=ex[:, 0:W]
                )

        # ---- transposed pass: o = attn @ vh ------------------------------
        o_sb = [osb_pool.tile([128, EH], F32, tag="osb", name=f"o_sb{j}") for j in range(NT)]
        for h in range(HH):
            et, hr = h // 2, (h % 2) * 64
            po = ps_o.tile([64, L], F32, tag="avo")
            for lkt in range(NT):
                base = lkt * 128
                Nlq = L - base
                ps = ps_s.tile([128, L], F32, tag="sc")
                for c0 in range(0, Nlq, 512):
                    N = min(512, Nlq - c0)
                    nc.tensor.matmul(
                        ps[:, c0:c0 + N],
                        lhsT=khT_t[et][hr:hr + 64, base:base + 128],
                        rhs=qhT_t[et][hr:hr + 64, base + c0:base + c0 + N],
                        start=True,
                        stop=True,
                    )
                nc.vector.tensor_add(ps[:, 0:128], ps[:, 0:128], trilT_t)
                ext = expt_pool.tile([128, L], F32, tag="expt")
                nc.scalar.activation(
                    out=ext[:, 0:Nlq], in_=ps[:, 0:Nlq], func=AF.Exp, scale=SCALE
                )
                # accumulate av into po over psum-bank-aligned spans
                if base < 512:
                    spans = [(base, 512), (512, L)]
                else:
                    spans = [(base, L)]
                for a, b2 in spans:
                    nc.tensor.matmul(
                        po[:, a:b2],
                        lhsT=vh_t[lkt][:, h * 64:(h + 1) * 64],
                        rhs=ext[:, a - base:b2 - base],
                        start=(lkt == 0),
                        stop=(lkt == (3 if b2 <= 512 else NT - 1)),
                        skip_group_check=True,
                    )
            ot = ot_pool.tile([64, L], F32, tag="ot")
            nc.scalar.copy(out=ot, in_=po[:, :])
            for j in range(NT):
                pt = ps_t.tile([128, 64], F32, tag="pt")
                nc.tensor.transpose(
                    pt, ot[:, j * 128:(j + 1) * 128], ident_t[0:64, 0:64]
                )
                nc.vector.tensor_scalar_mul(
                    o_sb[j][:, h * 64:(h + 1) * 64], pt, recs[j][:, h:h + 1]
                )
        for j in range(NT):
            nc.sync.dma_start(out=o_p[j * 128:(j + 1) * 128, :], in_=o_sb[j])

    _split_multiwaits(nc)
    return nc


def _build_ln_program():
    nc = bass.Bass()
    ROWS = (B * L) // N_CORES  # 512
    o_in = nc.dram_tensor("o_in", [ROWS, D], F32, kind="ExternalInput")
    q_in = nc.dram_tensor("q_in", [ROWS, D], F32, kind="ExternalInput")
    gamma = nc.dram_tensor("gamma", [1, D], F32, kind="ExternalInput")
    beta = nc.dram_tensor("beta", [1, D], F32, kind="ExternalInput")
    out_o = nc.dram_tensor("out_o", [ROWS, D], F32, kind="ExternalOutput")

    with ExitStack() as ctx:
        tc = ctx.enter_context(tile.TileContext(nc))
        const = ctx.enter_context(tc.tile_pool(name="const", bufs=1))
        pool = ctx.enter_context(tc.tile_pool(name="p", bufs=3))
        sp = ctx.enter_context(tc.tile_pool(name="sp", bufs=3))

        g_ap = gamma[:, :]
        g_t = const.tile([128, D], F32)
        nc.gpsimd.dma_start(
            out=g_t,
            in_=bass.AP(tensor=g_ap.tensor, offset=g_ap.offset, ap=[[0, 128]] + list(g_ap.ap[1:])),
        )
        b_ap = beta[:, :]
        b_t = const.tile([128, D], F32)
        nc.gpsimd.dma_start(
            out=b_t,
            in_=bass.AP(tensor=b_ap.tensor, offset=b_ap.offset, ap=[[0, 128]] + list(b_ap.ap[1:])),
        )
        eps_t = const.tile([128, 1], F32)
        nc.vector.memset(eps_t, LN_EPS)

        for t in range(ROWS // 128):
            sl = slice(t * 128, (t + 1) * 128)
            x = pool.tile([128, D], F32, tag="x")
            nc.sync.dma_start(out=x, in_=o_in[sl, :])
            qt = pool.tile([128, D], F32, tag="q")
            nc.sync.dma_start(out=qt, in_=q_in[sl, :])
            nc.vector.tensor_add(x, x, qt)
            st = sp.tile([128, 2, 6], F32, tag="st")
            for sg in range(2):
                nc.vector.bn_stats(
                    out=st[:, sg, :], in_=x[:, sg * 512:(sg + 1) * 512]
                )
            mv = sp.tile([128, 2], F32, tag="mv")
            nc.vector.bn_aggr(out=mv, in_=st)
            sd = sp.tile([128, 1], F32, tag="sd")
            nc.scalar.activation(
                out=sd, in_=mv[:, 1:2], func=AF.Sqrt, bias=eps_t, scale=1.0
            )
            nc.vector.reciprocal(sd, sd)
            y = pool.tile([128, D], F32, tag="y")
            nc.vector.tensor_scalar(
                out=y,
                in0=x,
                scalar1=mv[:, 0:1],
                scalar2=sd,
                op0=mybir.AluOpType.subtract,
                op1=mybir.AluOpType.mult,
            )
            nc.vector.tensor_mul(y, y, g_t)
            nc.vector.tensor_add(y, y, b_t)
            nc.sync.dma_start(out=out_o[sl, :], in_=y)

    _split_multiwaits(nc)
    return nc


_PROGS = {}
TRACE = False
LAST_RESULTS = {}


def _get_progs():
    if "attn" not in _PROGS:
        _PROGS["attn"] = _build_attn_program()
        _PROGS["ln"] = _build_ln_program()
    return _PROGS["attn"], _PROGS["ln"]


def kernel(q, k, v, mask, Wq, Wk, Wv, gamma, beta):
    q = np.ascontiguousarray(np.asarray(q, dtype=np.float32))
    k = np.asarray(k, dtype=np.float32)
    v = np.asarray(v, dtype=np.float32)
    Wq = np.asarray(Wq, dtype=np.float32)
    Wk = np.asarray(Wk, dtype=np.float32)
    Wv = np.asarray(Wv, dtype=np.float32)
    gamma = np.asarray(gamma, dtype=np.float32).reshape(1, D)
    beta = np.asarray(beta, dtype=np.float32).reshape(1, D)

    nc1, nc2 = _get_progs()

    ii, jj = np.meshgrid(np.arange(128), np.arange(128), indexing="ij")
    triu = np.where(jj > ii, np.float32(NEG), np.float32(0.0))
    trilT = np.where(ii > jj, np.float32(NEG), np.float32(0.0))
    ident = np.eye(128, dtype=np.float32)

    qTs = [np.ascontiguousarray(q[b].T) for b in range(B)]
    kTs = [np.ascontiguousarray(k[b].T) for b in range(B)]
    vTs = [np.ascontiguousarray(v[b].T) for b in range(B)]
    wqTs = [np.ascontiguousarray(Wq[s * EH:(s + 1) * EH].T) for s in range(2)]
    wkTs = [np.ascontiguousarray(Wk[s * EH:(s + 1) * EH].T) for s in range(2)]
    wvTs = [np.ascontiguousarray(Wv[s * EH:(s + 1) * EH].T) for s in range(2)]

    in_maps = []
    for c in range(N_CORES):
        b, s = c // 2, c % 2
        in_maps.append(
            {
                "qT": qTs[b],
                "kT": kTs[b],
                "vT": vTs[b],
                "wqT": wqTs[s],
                "wkT": wkTs[s],
                "wvT": wvTs[s],
                "triu": triu,
                "trilT": trilT,
                "ident": ident,
            }
        )
    res1 = bass_utils.run_bass_kernel_spmd(
        nc1, in_maps, core_ids=list(range(N_CORES)), trace=TRACE
    )
    LAST_RESULTS["attn"] = res1

    attn = np.empty((B, H, L, L), dtype=np.float32)
    o = np.empty((B, L, D), dtype=np.float32)
    for c in range(N_CORES):
        b, s = c // 2, c % 2
        attn[b, s * HH:(s + 1) * HH] = res1.results[c]["attn_p"]
        o[b, :, s * EH:(s + 1) * EH] = res1.results[c]["o_p"]

    ROWS = (B * L) // N_CORES
    in_maps2 = []
    for c in range(N_CORES):
        b, half = c // 2, c % 2
        sl = slice(half * ROWS, (half + 1) * ROWS)
        in_maps2.append(
            {
                "o_in": np.ascontiguousarray(o[b, sl]),
                "q_in": np.ascontiguousarray(q[b, sl]),
                "gamma": gamma,
                "beta": beta,
            }
        )
    res2 = bass_utils.run_bass_kernel_spmd(
        nc2, in_maps2, core_ids=list(range(N_CORES)), trace=TRACE
    )
    LAST_RESULTS["ln"] = res2

    out = np.empty((B, L, D), dtype=np.float32)
    for c in range(N_CORES):
        b, half = c // 2, c % 2
        sl = slice(half * ROWS, (half + 1) * ROWS)
        out[b, sl] = res2.results[c]["out_o"]

    return out, attn


# revision 9
# speedup vs baseline: 1.5852x; 1.5852x over previous
"""Trainium2 Bass kernel for multi-head attention + residual + LayerNorm.

Problem: B=4, L=1024, D=1024, H=16, DK=DV=64, causal mask, returns
(out, attn) like the reference:
    qh = q @ Wq.T (split heads), kh = k @ Wk.T, vh = v @ Wv.T
    attn = softmax(mask(qh kh^T / sqrt(DK)))
    o = attn @ vh  (merge heads) + q ; out = LayerNorm(o) * gamma + beta

Sharding (8 cores): kernel 1 is head-parallel — core c handles batch c//2
and heads (c%2)*8..(c%2)*8+8, full query range. Every core runs an
identical program (causal structure is the same on all cores). Kernel 2
does residual+LayerNorm, row-parallel (512 rows per core).

Layout strategy (per core, kernel 1):
  - host passes transposed qT/kT/vT [D, L] and wqT/wkT/wvT [D, 512] so all
    projection matmuls contract d on partitions.
  - projections produce qhT/khT [e, lq] (e on partitions) and vh [lk, e].
  - natural scores [lq, lk] per (head, lq-tile): K=64 matmul from qhT/khT
    slices; causal -1e9 bias added on the diagonal 128-block; exp on
    ScalarE with fused row-sum (accum_out); normalize with per-partition
    reciprocal via tensor_scalar; DMA straight to the attn output.
  - transposed scores [lk, lq] are recomputed (cheaper than transposing
    attn through PSUM) for the attn^T @ v contraction; o comes out as
    oT [64, lq] accumulated over lk tiles, PE-transposed back and scaled
    by the same reciprocals.
Causality is exploited everywhere: upper-triangle blocks are never
computed; the attn output buffer is pre-zeroed by the runtime.
"""

import math
from contextlib import ExitStack

import ml_dtypes
import numpy as np

import concourse.bass as bass
import concourse.tile as tile
from concourse import mybir
import concourse.bass_utils as bass_utils

F32 = mybir.dt.float32
BF16 = mybir.dt.bfloat16
MM_DT = BF16  # dtype for matmul operands (projections, scores, av)
AF = mybir.ActivationFunctionType

B = 4
L = 1024
D = 1024
H = 16
DK = 64
HH = H // 2          # heads per core (head-half)
EH = HH * DK         # 512 output dims per core
NT = L // 128        # 8 row tiles
ND = D // 128        # 8 contraction tiles
NE = EH // 128       # 4 e-tiles per core
N_CORES = 8
NEG = -1.0e9
LN_EPS = 1e-5
SCALE = 1.0 / math.sqrt(DK)


def _split_multiwaits(nc, max_waits=1):
    """walrus in this toolchain rejects >1 sync-wait per instruction; move
    extra waits onto NoOps inserted right before (same engine, same order —
    semantically identical)."""
    uid = [0]
    for func in nc.m.functions:
        for block in func.blocks:
            new_insts = []
            for inst in block.instructions:
                si = inst.sync_info
                if si is not None and si.on_wait and len(si.on_wait) > max_waits:
                    waits = list(si.on_wait)
                    for w in waits[:-max_waits]:
                        uid[0] += 1
                        nop = mybir.InstNoOp(
                            name=f"I-waitsplit-{uid[0]}",
                            engine=inst.engine,
                            sync_info=mybir.SyncInfo(on_wait=[w], on_update=[]),
                            ins=[],
                            outs=[],
                        )
                        new_insts.append(nop)
                        nc.inst_map[nop.name] = nop
                    inst.sync_info = mybir.SyncInfo(
                        on_wait=waits[-max_waits:], on_update=list(si.on_update or [])
                    )
                new_insts.append(inst)
            block.instructions[:] = new_insts


def _build_attn_program():
    nc = bass.Bass()
    qT = nc.dram_tensor("qT", [D, L], MM_DT, kind="ExternalInput")
    kT = nc.dram_tensor("kT", [D, L], MM_DT, kind="ExternalInput")
    vT = nc.dram_tensor("vT", [D, L], MM_DT, kind="ExternalInput")
    wqT = nc.dram_tensor("wqT", [D, EH], MM_DT, kind="ExternalInput")
    wkT = nc.dram_tensor("wkT", [D, EH], MM_DT, kind="ExternalInput")
    wvT = nc.dram_tensor("wvT", [D, EH], MM_DT, kind="ExternalInput")
    triu = nc.dram_tensor("triu", [128, 128], F32, kind="ExternalInput")
    trilT = nc.dram_tensor("trilT", [128, 128], F32, kind="ExternalInput")
    ident = nc.dram_tensor("ident", [128, 128], F32, kind="ExternalInput")
    attn_p = nc.dram_tensor("attn_p", [HH, L, L], F32, kind="ExternalOutput")
    o_p = nc.dram_tensor("o_p", [L, EH], F32, kind="ExternalOutput")

    with ExitStack() as ctx:
        tc = ctx.enter_context(tile.TileContext(nc))
        const = ctx.enter_context(tc.tile_pool(name="const", bufs=1))
        w_pool = ctx.enter_context(tc.tile_pool(name="w", bufs=9))
        x_pool = ctx.enter_context(tc.tile_pool(name="x", bufs=9))
        qhT_pool = ctx.enter_context(tc.tile_pool(name="qhT", bufs=NE))
        khT_pool = ctx.enter_context(tc.tile_pool(name="khT", bufs=NE))
        vh_pool = ctx.enter_context(tc.tile_pool(name="vh", bufs=NT))
        expn_pool = ctx.enter_context(tc.tile_pool(name="expn", bufs=HH + 1))
        expt_pool = ctx.enter_context(tc.tile_pool(name="expt", bufs=3))
        osb_pool = ctx.enter_context(tc.tile_pool(name="osb", bufs=NT))
        ot_pool = ctx.enter_context(tc.tile_pool(name="ot", bufs=2))
        stat_pool = ctx.enter_context(tc.tile_pool(name="stat", bufs=2))
        rec_pool = ctx.enter_context(tc.tile_pool(name="rec", bufs=NT))
        ps_s = ctx.enter_context(tc.tile_pool(name="ps_s", bufs=2, space="PSUM"))
        ps_o = ctx.enter_context(tc.tile_pool(name="ps_o", bufs=1, space="PSUM"))
        ps_t = ctx.enter_context(tc.tile_pool(name="ps_t", bufs=2, space="PSUM"))

        triu_t = const.tile([128, 128], F32)
        nc.sync.dma_start(out=triu_t, in_=triu[:, :])
        trilT_t = const.tile([128, 128], F32)
        nc.sync.dma_start(out=trilT_t, in_=trilT[:, :])
        ident_t = const.tile([128, 128], F32)
        nc.sync.dma_start(out=ident_t, in_=ident[:, :])

        # ---- projections -------------------------------------------------
        def project(w_dram, x_dram, n_out_tiles, transposed_out):
            """transposed_out=True: produce [e, l] tiles (accumulate lhsT=w
            slices); False: produce [l_tile, e] tiles (lhsT=x slices)."""
            wt = []
            xt = []
            for dt in range(ND):
                w_t = w_pool.tile([128, EH], MM_DT, tag="w")
                nc.sync.dma_start(out=w_t, in_=w_dram[dt * 128:(dt + 1) * 128, :])
                wt.append(w_t)
                x_t = x_pool.tile([128, L], MM_DT, tag="x")
                nc.sync.dma_start(out=x_t, in_=x_dram[dt * 128:(dt + 1) * 128, :])
                xt.append(x_t)
            out_tiles = []
            if transposed_out:
                # out[e_tile][128, L]
                for et in range(NE):
                    ot = None
                    for lqc in range(L // 512):
                        ps = ps_s.tile([128, L], F32, tag="sc")
                        for dt in range(ND):
                            nc.tensor.matmul(
                                ps[:, 0:512],
                                lhsT=wt[dt][:, et * 128:(et + 1) * 128],
                                rhs=xt[dt][:, lqc * 512:(lqc + 1) * 512],
                                start=(dt == 0),
                                stop=(dt == ND - 1),
                            )
                        if ot is None:
                            ot = qhT_pool.tile([128, L], MM_DT, tag=transposed_out)
                        nc.scalar.copy(
                            out=ot[:, lqc * 512:(lqc + 1) * 512], in_=ps[:, 0:512]
                        )
                    out_tiles.append(ot)
            else:
                # out[lk_tile][128, EH]
                for lkt in range(NT):
                    ps = ps_s.tile([128, L], F32, tag="sc")
                    for dt in range(ND):
                        nc.tensor.matmul(
                            ps[:, 0:EH],
                            lhsT=xt[dt][:, lkt * 128:(lkt + 1) * 128],
                            rhs=wt[dt][:, 0:EH],
                            start=(dt == 0),
                            stop=(dt == ND - 1),
                        )
                    vt = vh_pool.tile([128, EH], MM_DT, tag="vh")
                    nc.scalar.copy(out=vt, in_=ps[:, 0:EH])
                    out_tiles.append(vt)
            return out_tiles

        qhT_t = project(wqT, qT, NE, "qhT")
        khT_t = project(wkT, kT, NE, "khT")
        vh_t = project(wvT, vT, NT, False)

        # ---- natural-layout pass: attn output + row sums ----------------
        recs = []
        exp_nat = {}
        for t in range(NT):
            W = 128 * (t + 1)
            sum_t = stat_pool.tile([128, HH], F32, tag="sums")
            exp_tiles = []
            for h in range(HH):
                et, hr = h // 2, (h % 2) * 64
                ps = ps_s.tile([128, L], F32, tag="sc")
                for c0 in range(0, W, 512):
                    N = min(512, W - c0)
                    nc.tensor.matmul(
                        ps[:, c0:c0 + N],
                        lhsT=qhT_t[et][hr:hr + 64, t * 128:(t + 1) * 128],
                        rhs=khT_t[et][hr:hr + 64, c0:c0 + N],
                        start=True,
                        stop=True,
                    )
                nc.vector.tensor_add(
                    ps[:, t * 128:(t + 1) * 128],
                    ps[:, t * 128:(t + 1) * 128],
                    triu_t,
                )
                ex = expn_pool.tile([128, L], F32, tag="expn")
                nc.scalar.activation(
                    out=ex[:, 0:W],
                    in_=ps[:, 0:W],
                    func=AF.Exp,
                    scale=SCALE,
                    accum_out=sum_t[:, h:h + 1],
                )
                exp_tiles.append(ex)
            rec_t = rec_pool.tile([128, HH], F32, tag="rec")
            nc.vector.reciprocal(rec_t, sum_t)
            recs.append(rec_t)
            for h in range(HH):
                ex = exp_tiles[h]
                nc.vector.tensor_scalar_mul(ex[:, 0:W], ex[:, 0:W], rec_t[:, h:h + 1])
                nc.sync.dma_start(
                    out=attn_p[h, t * 128:(t + 1) * 128, 0:W], in_=ex[:, 0:W]
                )

        # ---- transposed pass: o = attn @ vh ------------------------------
        o_sb = [osb_pool.tile([128, EH], F32, tag="osb", name=f"o_sb{j}") for j in range(NT)]
        for h in range(HH):
            et, hr = h // 2, (h % 2) * 64
            po = ps_o.tile([64, L], F32, tag="avo")
            for lkt in range(NT):
                base = lkt * 128
                Nlq = L - base
                ps = ps_s.tile([128, L], F32, tag="sc")
                for c0 in range(0, Nlq, 512):
                    N = min(512, Nlq - c0)
                    nc.tensor.matmul(
                        ps[:, c0:c0 + N],
                        lhsT=khT_t[et][hr:hr + 64, base:base + 128],
                        rhs=qhT_t[et][hr:hr + 64, base + c0:base + c0 + N],
                        start=True,
                        stop=True,
                    )
                nc.vector.tensor_add(ps[:, 0:128], ps[:, 0:128], trilT_t)
                ext = expt_pool.tile([128, L], MM_DT, tag="expt")
                nc.scalar.activation(
                    out=ext[:, 0:Nlq], in_=ps[:, 0:Nlq], func=AF.Exp, scale=SCALE
                )
                # accumulate av into po over psum-bank-aligned spans
                if base < 512:
                    spans = [(base, 512), (512, L)]
                else:
                    spans = [(base, L)]
                for a, b2 in spans:
                    nc.tensor.matmul(
                        po[:, a:b2],
                        lhsT=vh_t[lkt][:, h * 64:(h + 1) * 64],
                        rhs=ext[:, a - base:b2 - base],
                        start=(lkt == 0),
                        stop=(lkt == (3 if b2 <= 512 else NT - 1)),
                        skip_group_check=True,
                    )
            ot = ot_pool.tile([64, L], F32, tag="ot")
            nc.scalar.copy(out=ot, in_=po[:, :])
            for j in range(NT):
                pt = ps_t.tile([128, 64], F32, tag="pt")
                nc.tensor.transpose(
                    pt, ot[:, j * 128:(j + 1) * 128], ident_t[0:64, 0:64]
                )
                nc.vector.tensor_scalar_mul(
                    o_sb[j][:, h * 64:(h + 1) * 64], pt, recs[j][:, h:h + 1]
                )
        for j in range(NT):
            nc.sync.dma_start(out=o_p[j * 128:(j + 1) * 128, :], in_=o_sb[j])

    _split_multiwaits(nc)
    return nc


def _build_ln_program():
    nc = bass.Bass()
    ROWS = (B * L) // N_CORES  # 512
    o_in = nc.dram_tensor("o_in", [ROWS, D], F32, kind="ExternalInput")
    q_in = nc.dram_tensor("q_in", [ROWS, D], F32, kind="ExternalInput")
    gamma = nc.dram_tensor("gamma", [1, D], F32, kind="ExternalInput")
    beta = nc.dram_tensor("beta", [1, D], F32, kind="ExternalInput")
    out_o = nc.dram_tensor("out_o", [ROWS, D], F32, kind="ExternalOutput")

    with ExitStack() as ctx:
        tc = ctx.enter_context(tile.TileContext(nc))
        const = ctx.enter_context(tc.tile_pool(name="const", bufs=1))
        pool = ctx.enter_context(tc.tile_pool(name="p", bufs=3))
        sp = ctx.enter_context(tc.tile_pool(name="sp", bufs=3))

        g_ap = gamma[:, :]
        g_t = const.tile([128, D], F32)
        nc.gpsimd.dma_start(
            out=g_t,
            in_=bass.AP(tensor=g_ap.tensor, offset=g_ap.offset, ap=[[0, 128]] + list(g_ap.ap[1:])),
        )
        b_ap = beta[:, :]
        b_t = const.tile([128, D], F32)
        nc.gpsimd.dma_start(
            out=b_t,
            in_=bass.AP(tensor=b_ap.tensor, offset=b_ap.offset, ap=[[0, 128]] + list(b_ap.ap[1:])),
        )
        eps_t = const.tile([128, 1], F32)
        nc.vector.memset(eps_t, LN_EPS)

        for t in range(ROWS // 128):
            sl = slice(t * 128, (t + 1) * 128)
            x = pool.tile([128, D], F32, tag="x")
            nc.sync.dma_start(out=x, in_=o_in[sl, :])
            qt = pool.tile([128, D], F32, tag="q")
            nc.sync.dma_start(out=qt, in_=q_in[sl, :])
            nc.vector.tensor_add(x, x, qt)
            st = sp.tile([128, 2, 6], F32, tag="st")
            for sg in range(2):
                nc.vector.bn_stats(
                    out=st[:, sg, :], in_=x[:, sg * 512:(sg + 1) * 512]
                )
            mv = sp.tile([128, 2], F32, tag="mv")
            nc.vector.bn_aggr(out=mv, in_=st)
            sd = sp.tile([128, 1], F32, tag="sd")
            nc.scalar.activation(
                out=sd, in_=mv[:, 1:2], func=AF.Sqrt, bias=eps_t, scale=1.0
            )
            nc.vector.reciprocal(sd, sd)
            y = pool.tile([128, D], F32, tag="y")
            nc.vector.tensor_scalar(
                out=y,
                in0=x,
                scalar1=mv[:, 0:1],
                scalar2=sd,
                op0=mybir.AluOpType.subtract,
                op1=mybir.AluOpType.mult,
            )
            nc.vector.tensor_mul(y, y, g_t)
            nc.vector.tensor_add(y, y, b_t)
            nc.sync.dma_start(out=out_o[sl, :], in_=y)

    _split_multiwaits(nc)
    return nc


_PROGS = {}
TRACE = False
LAST_RESULTS = {}


def _get_progs():
    if "attn" not in _PROGS:
        _PROGS["attn"] = _build_attn_program()
        _PROGS["ln"] = _build_ln_program()
    return _PROGS["attn"], _PROGS["ln"]


def kernel(q, k, v, mask, Wq, Wk, Wv, gamma, beta):
    q = np.ascontiguousarray(np.asarray(q, dtype=np.float32))
    k = np.asarray(k, dtype=np.float32)
    v = np.asarray(v, dtype=np.float32)
    Wq = np.asarray(Wq, dtype=np.float32)
    Wk = np.asarray(Wk, dtype=np.float32)
    Wv = np.asarray(Wv, dtype=np.float32)
    gamma = np.asarray(gamma, dtype=np.float32).reshape(1, D)
    beta = np.asarray(beta, dtype=np.float32).reshape(1, D)

    nc1, nc2 = _get_progs()

    ii, jj = np.meshgrid(np.arange(128), np.arange(128), indexing="ij")
    triu = np.where(jj > ii, np.float32(NEG), np.float32(0.0))
    trilT = np.where(ii > jj, np.float32(NEG), np.float32(0.0))
    ident = np.eye(128, dtype=np.float32)

    mdt = ml_dtypes.bfloat16 if MM_DT == BF16 else np.float32
    qTs = [np.ascontiguousarray(q[b].T.astype(mdt)) for b in range(B)]
    kTs = [np.ascontiguousarray(k[b].T.astype(mdt)) for b in range(B)]
    vTs = [np.ascontiguousarray(v[b].T.astype(mdt)) for b in range(B)]
    wqTs = [np.ascontiguousarray(Wq[s * EH:(s + 1) * EH].T.astype(mdt)) for s in range(2)]
    wkTs = [np.ascontiguousarray(Wk[s * EH:(s + 1) * EH].T.astype(mdt)) for s in range(2)]
    wvTs = [np.ascontiguousarray(Wv[s * EH:(s + 1) * EH].T.astype(mdt)) for s in range(2)]

    in_maps = []
    for c in range(N_CORES):
        b, s = c // 2, c % 2
        in_maps.append(
            {
                "qT": qTs[b],
                "kT": kTs[b],
                "vT": vTs[b],
                "wqT": wqTs[s],
                "wkT": wkTs[s],
                "wvT": wvTs[s],
                "triu": triu,
                "trilT": trilT,
                "ident": ident,
            }
        )
    res1 = bass_utils.run_bass_kernel_spmd(
        nc1, in_maps, core_ids=list(range(N_CORES)), trace=TRACE
    )
    LAST_RESULTS["attn"] = res1

    attn = np.empty((B, H, L, L), dtype=np.float32)
    o = np.empty((B, L, D), dtype=np.float32)
    for c in range(N_CORES):
        b, s = c // 2, c % 2
        attn[b, s * HH:(s + 1) * HH] = res1.results[c]["attn_p"]
        o[b, :, s * EH:(s + 1) * EH] = res1.results[c]["o_p"]

    ROWS = (B * L) // N_CORES
    in_maps2 = []
    for c in range(N_CORES):
        b, half = c // 2, c % 2
        sl = slice(half * ROWS, (half + 1) * ROWS)
        in_maps2.append(
            {
                "o_in": np.ascontiguousarray(o[b, sl]),
                "q_in": np.ascontiguousarray(q[b, sl]),
                "gamma": gamma,
                "beta": beta,
            }
        )
    res2 = bass_utils.run_bass_kernel_spmd(
        nc2, in_maps2, core_ids=list(range(N_CORES)), trace=TRACE
    )
    LAST_RESULTS["ln"] = res2

    out = np.empty((B, L, D), dtype=np.float32)
    for c in range(N_CORES):
        b, half = c // 2, c % 2
        sl = slice(half * ROWS, (half + 1) * ROWS)
        out[b, sl] = res2.results[c]["out_o"]

    return out, attn
